# revision 16
# baseline (speedup 1.0000x reference)
"""Trainium2 Bass kernel for the rank-weighted log-loss reduction.

loss = -sum_i ri * (log(p_win_i) - R*(f0_i - P1)^2),  ri = i / (n*(n+1)/2)

Strategy (pure data parallel over 8 cores, memory-bound -> minimize and
smooth HBM traffic):
  - core k gets rows [k*M, (k+1)*M), M = N/8.
  - host packs inputs for the wire: probabilities to bf16 (2x fewer
    bytes; the on-chip math is bf16 anyway and the result stays ~1e-4
    relative), winner indices to int8 pre-transposed to [P, T*F] so all
    of pv loads in four 512 KiB DMAs and stays resident in SBUF.
  - fo streams in on the SP HWDGE as five 2-tile (4 MiB-of-bf16) groups,
    then singles/halves/quarters tapering down, every X buffer written
    exactly once (no recycle waits; the Tile framework only has 8 HWDGE
    completion-semaphore lanes, so small late dma_starts would otherwise
    stall behind 2-tile predecessors).
  - per tile: predicated copy overwrites the f0 column with f1 IN PLACE
    (no separate p_win copy); (f0-P1)^2 is split — half columns as two
    DVE bf16 ops, half on the scalar engine — so no engine exceeds the
    DMA rate; Ln on the scalar engine reads the predicated column.
  - matmuls with per-tile stationary columns (1, lo, hi) and their
    NEGATION fold +log(p) and -(f0-P1)^2 into one fp32 PSUM accumulator;
    pos = 128*t + p, and the lo/hi byte split keeps weights exact in
    bf16.
  - the last tile is 4 column-chunks so only a ~256-column chain is
    exposed after the final DMA; its PSUM columns copy out per chunk and
    the output DMA goes in two halves.
  - host folds the per-core [3, F] partials into the closed-form
    weighted sum (weights are affine in (pos, f)) in float64.
"""

import numpy as np
import ml_dtypes
from contextlib import ExitStack

import concourse.bass as bass
import concourse.mybir as mybir
import concourse.tile as tile
from concourse.bass_utils import run_bass_kernel_spmd


MAX_SYNC_WAITS = 1


def _spill_excess_waits(nc, max_waits=MAX_SYNC_WAITS):
    """The walrus in this toolchain rejects instructions carrying more than
    a couple of sync waits ("Too many sync wait commands"). Spill the excess
    onto same-engine NOPs inserted immediately before — semantically
    identical (consecutive sem-ge waits on one engine)."""
    import bass_rust

    k = 0
    for f in nc.m.functions:
        for b in f.blocks:
            out = []
            changed = False
            for inst in b.instructions:
                si = inst.sync_info
                waits = list(si.on_wait or []) if si is not None else []
                if len(waits) > max_waits:
                    chunks = [
                        waits[i : i + max_waits]
                        for i in range(0, len(waits), max_waits)
                    ]
                    for chunk in chunks[:-1]:
                        nop = mybir.InstNoOp(name=f"antspill-{k}", ins=[], outs=[])
                        k += 1
                        nop.engine = inst.engine
                        nop.sync_info = bass_rust.SyncInfo(
                            on_wait=chunk, on_update=[]
                        )
                        out.append(nop)
                    inst.sync_info = bass_rust.SyncInfo(
                        on_wait=chunks[-1], on_update=list(si.on_update or [])
                    )
                    changed = True
                out.append(inst)
            if changed:
                b.instructions = out

N_TOTAL = 16777216
N_CORES = 8
P = 128          # SBUF partitions
F = 1024         # rows per partition per tile
T = 16           # tiles per core; P*F*T = 2097152 = N_TOTAL/N_CORES
CH = 4           # column chunks for the tapered last tile
CF = F // CH
R = 1.0
P1 = 0.5


def build_nc(F=F, T=T):
    M = P * F * T
    nc = bass.Bass(
        "TRN2", target_bir_lowering=False, debug=False,
        enable_asserts=False, num_devices=1,
    )
    fo = nc.dram_tensor("fo", [M, 2], mybir.dt.bfloat16, kind="ExternalInput")
    pv = nc.dram_tensor("pv", [P, T * F], mybir.dt.int8, kind="ExternalInput")
    wt = nc.dram_tensor("wt", [P, 6 * T], mybir.dt.bfloat16, kind="ExternalInput")
    out = nc.dram_tensor("out", [3, F], mybir.dt.float32, kind="ExternalOutput")

    fo_r = fo.ap().rearrange("(t p f) c -> t p f c", t=T, p=P, f=F)
    # 2-tile DMA groups: per partition two contiguous 4 KiB chunks. Fewer,
    # larger dma_starts keep more bytes in flight per HWDGE semaphore lane
    # (the Tile framework rotates over only 8 of them).
    fo_g = fo.ap().rearrange("(u v p f) c -> u p v f c", u=T // 2, v=2, p=P, f=F)

    with tile.TileContext(nc) as tc, ExitStack() as ctx:
        xp = ctx.enter_context(tc.tile_pool(name="xp", bufs=4))
        mp = ctx.enter_context(tc.tile_pool(name="mp", bufs=4))
        cp = ctx.enter_context(tc.tile_pool(name="cp", bufs=1))
        ps = ctx.enter_context(tc.tile_pool(name="ps", bufs=1, space="PSUM"))

        nbias = cp.tile([P, 1], mybir.dt.float32)
        nc.vector.memset(nbias[:], -P1)
        W = cp.tile([P, 6 * T], mybir.dt.bfloat16)
        nc.scalar.dma_start(W[:], wt[:])
        V_all = cp.tile([P, T * F], mybir.dt.int8)
        # four quarter-loads on the otherwise-idle Activation HWDGE: the
        # first quarter lands fast (doesn't sit behind 16 KiB descriptor
        # trains), unblocking the first tiles' predicated copies early
        pv_q = pv.ap().rearrange("p (q c) -> q p c", q=4)
        for q in range(4):
            nc.scalar.dma_start(V_all[:, q * (T * F // 4) : (q + 1) * (T * F // 4)], pv_q[q])
        acc = ps.tile([3, F], mybir.dt.float32)
        ob = cp.tile([3, F], mybir.dt.float32)

        def chunk_load(t, h0, w, suf, bufs):
            """DMA + Square + in-place predicated copy for a column chunk."""
            sl = slice(h0, h0 + w)
            Xc = xp.tile([P, w, 2], mybir.dt.bfloat16, tag=f"Xc{suf}", bufs=bufs,
                         name=f"Xc_{t}_{h0}")
            nc.sync.dma_start(Xc[:], fo_r[t, :, sl])
            sqc = mp.tile([P, w], mybir.dt.bfloat16, tag=f"sqc{suf}", bufs=bufs,
                          name=f"sqc_{t}_{h0}")
            nc.scalar.activation(
                sqc[:], Xc[:, :, 0], mybir.ActivationFunctionType.Square,
                bias=nbias[:],
            )
            nc.vector.copy_predicated(
                Xc[:, :, 0], V_all[:, t * F + h0 : t * F + h0 + w], Xc[:, :, 1]
            )
            return Xc, sqc

        def chunk_ln(t, h0, w, suf, bufs, Xc):
            lpc = mp.tile([P, w], mybir.dt.bfloat16, tag=f"lpc{suf}", bufs=bufs,
                          name=f"lpc_{t}_{h0}")
            nc.scalar.activation(lpc[:], Xc[:, :, 0], mybir.ActivationFunctionType.Ln)
            return lpc

        # full tiles 0..T-7 in groups of 2 per dma_start
        def full_tile(t, Xv, Xsq):
            """p_win in place: overwrite the f0 column with f1 where the
            winner index is 1. The squares read f0 first (WAR dep inserted
            by the tile framework); Ln then reads the predicated column."""
            nc.vector.copy_predicated(
                Xv, V_all[:, t * F : (t + 1) * F], Xsq
            )
            return Xv

        def square_split(t, Xf0):
            """(f0-P1)^2 -> bf16; cols [0:HS) via two DVE bf16 ops, the rest
            on the scalar engine, so neither engine exceeds the DMA rate."""
            HS = 512
            sq = mp.tile([P, F], mybir.dt.bfloat16, tag="sq", bufs=6,
                         name=f"sq_{t}")
            tmp = mp.tile([P, HS], mybir.dt.bfloat16, tag="sqtmp", bufs=4,
                          name=f"sqtmp_{t}")
            nc.vector.tensor_scalar_add(tmp[:], Xf0[:, 0:HS], -P1)
            nc.vector.tensor_tensor(
                sq[:, 0:HS], tmp[:], tmp[:], op=mybir.AluOpType.mult
            )
            nc.scalar.activation(
                sq[:, HS:F], Xf0[:, HS:F],
                mybir.ActivationFunctionType.Square, bias=nbias[:],
            )
            return sq

        def tile_tail(t, pw, sq):
            lp = mp.tile([P, F], mybir.dt.bfloat16, tag="lp", bufs=4, name=f"lp_{t}")
            nc.scalar.activation(lp[:], pw, mybir.ActivationFunctionType.Ln)
            for h0 in range(0, F, 512):
                sl = slice(h0, h0 + 512)
                nc.tensor.matmul(
                    acc[:, sl], W[:, 6 * t : 6 * t + 3], lp[:, sl],
                    start=(t == 0), stop=False,
                )
                nc.tensor.matmul(
                    acc[:, sl], W[:, 6 * t + 3 : 6 * t + 6], sq[:, sl],
                    start=False, stop=False,
                )

        pending = []

        def schedule(tail_fn):
            # keep exactly one unit of load-phase lookahead
            if pending:
                pending.pop(0)()
            pending.append(tail_fn)

        NG = 5   # groups of 2 full tiles: t0..t9 (t10..15 taper down)
        for g in range(NG):
            X2 = xp.tile([P, 2, F, 2], mybir.dt.bfloat16, tag="X2", bufs=5)
            nc.sync.dma_start(X2[:], fo_g[g])
            sqs = [
                square_split(2 * g + v, X2[:, v, :, 0]) for v in range(2)
            ]
            pws = [
                full_tile(2 * g + v, X2[:, v, :, 0], X2[:, v, :, 1])
                for v in range(2)
            ]

            def group_tail(g=g, pws=pws, sqs=sqs):
                for v in range(2):
                    tile_tail(2 * g + v, pws[v], sqs[v])

            schedule(group_tail)

        # taper: singles t10..13 (own buffers, no recycling)
        for t in range(2 * NG, T - 2):
            X = xp.tile([P, F, 2], mybir.dt.bfloat16, tag="X1", bufs=4,
                        name=f"X_{t}")
            nc.sync.dma_start(X[:], fo_r[t])
            sq = square_split(t, X[:, :, 0])
            pw = full_tile(t, X[:, :, 0], X[:, :, 1])
            schedule(lambda t=t, pw=pw, sq=sq: tile_tail(t, pw, sq))

        # halves of t14
        t = T - 2
        def half_tail(t, h0, Xc, sqc):
            sl = slice(h0, h0 + 512)
            lpc = chunk_ln(t, h0, 512, "h", 2, Xc)
            nc.tensor.matmul(
                acc[:, sl], W[:, 6 * t : 6 * t + 3], lpc[:],
                start=False, stop=False,
            )
            nc.tensor.matmul(
                acc[:, sl], W[:, 6 * t + 3 : 6 * t + 6], sqc[:],
                start=False, stop=False,
            )
        for h0 in range(0, F, 512):
            Xc, sqc = chunk_load(t, h0, 512, "h", 2)
            schedule(lambda t=t, h0=h0, Xc=Xc, sqc=sqc: half_tail(t, h0, Xc, sqc))

        # quarter chunks of the last tile: they land last, with only a
        # ~256-column chain exposed behind them
        t = T - 1
        def quarter_tail(t, c, Xc, sqc):
            h0 = c * CF
            sl = slice(h0, h0 + CF)
            lpc = chunk_ln(t, h0, CF, "q", CH, Xc)
            nc.tensor.matmul(
                acc[:, sl], W[:, 6 * t : 6 * t + 3], lpc[:],
                start=False, stop=False,
            )
            nc.tensor.matmul(
                acc[:, sl], W[:, 6 * t + 3 : 6 * t + 6], sqc[:],
                start=False, stop=True,
            )
            nc.vector.tensor_copy(ob[:, sl], acc[:, sl])
            if c == 1:
                nc.sync.dma_start(out[:, 0:512], ob[:, 0:512])
        for c in range(CH):
            Xc, sqc = chunk_load(t, c * CF, CF, "q", CH)
            schedule(lambda t=t, c=c, Xc=Xc, sqc=sqc: quarter_tail(t, c, Xc, sqc))
        while pending:
            pending.pop(0)()
        nc.sync.dma_start(out[:, 512:F], ob[:, 512:F])
    _spill_excess_waits(nc)
    return nc


def build_wt(T=T):
    """Per-tile stationary matrix: columns (1, pos_lo, pos_hi, -1, -pos_lo,
    -pos_hi), where pos = 128*t + p is the row-chunk index. The negated
    triple multiplies the Square term so PSUM performs log(p) - (f0-P1)^2.
    lo/hi split keeps values exact in bf16 (lo < 256; hi a multiple of 256)."""
    cols = np.zeros((P, 6 * T), np.float32)
    p_idx = np.arange(P, dtype=np.int64)
    for t in range(T):
        pos = t * P + p_idx
        lo = pos & 255
        hi = pos - lo
        cols[:, 6 * t] = 1.0
        cols[:, 6 * t + 1] = lo
        cols[:, 6 * t + 2] = hi
        cols[:, 6 * t + 3] = -1.0
        cols[:, 6 * t + 4] = -lo
        cols[:, 6 * t + 5] = -hi
    return cols.astype(ml_dtypes.bfloat16)


def combine(outs, F=F, T=T):
    """Fold per-core [3, F] partials into the loss.

    Row i = k*M + pos*F + f. Per core:
      sum_i per_i * i = k*M*S + F*(sum pos*per) + (sum f*per)
    with S = sum(c0), sum pos*per = sum(c_lo + c_hi), sum f*per = sum(f*c0).
    """
    M = P * F * T
    n = M * len(outs)
    # mirror the reference's fp32 denom computation
    denom = float(np.float32(n) * np.float32(n + 1) * np.float32(0.5))
    j = np.arange(F, dtype=np.float64)
    total = 0.0
    for k, o in enumerate(outs):
        c0 = o[0].astype(np.float64)
        cw = o[1].astype(np.float64) + o[2].astype(np.float64)
        total += (k * M) * c0.sum() + F * cw.sum() + (j * c0).sum()
    return -total / denom


_NC_CACHE = {}


def _run(final_out, point_victor, **spmd_kwargs):
    fo = np.ascontiguousarray(np.asarray(final_out, dtype=np.float32).astype(ml_dtypes.bfloat16))
    pv = np.asarray(point_victor)
    assert fo.shape == (N_TOTAL, 2) and pv.shape == (N_TOTAL,)
    M = N_TOTAL // N_CORES
    # pack the 0/1 winner indices to int8 and pre-transpose per core to
    # [P, T*F] (partition-major) so the kernel loads all of pv in one DMA
    # with 16 KiB descriptors — 4x less HBM traffic than int32
    pv8 = pv.astype(np.int8).reshape(N_CORES, T, P, F).transpose(0, 2, 1, 3)
    pv8 = np.ascontiguousarray(pv8).reshape(N_CORES, P, T * F)

    if "nc" not in _NC_CACHE:
        _NC_CACHE["nc"] = build_nc()
    nc = _NC_CACHE["nc"]
    wt = build_wt()

    in_maps = [
        {"fo": fo[k * M : (k + 1) * M], "pv": pv8[k], "wt": wt}
        for k in range(N_CORES)
    ]
    res = run_bass_kernel_spmd(nc, in_maps, core_ids=list(range(N_CORES)), **spmd_kwargs)
    outs = [r["out"] for r in res.results]
    return np.float32(combine(outs)), res


def kernel(final_out, point_victor):
    return _run(final_out, point_victor)[0]



# revision 17
# speedup vs baseline: 1.1486x; 1.1486x over previous
"""Trainium2 Bass kernel for the rank-weighted log-loss reduction.

loss = -sum_i ri * (log(p_win_i) - R*(f0_i - P1)^2),  ri = i / (n*(n+1)/2)

Strategy (pure data parallel over 8 cores, memory-bound -> minimize and
smooth HBM traffic):
  - core k gets rows [k*M, (k+1)*M), M = N/8.
  - host packs inputs for the wire: probabilities to bf16 (2x fewer
    bytes; the on-chip math is bf16 anyway and the result stays ~1e-4
    relative), winner indices to int8 pre-transposed to [P, T*F] so all
    of pv loads in four 512 KiB DMAs and stays resident in SBUF.
  - fo streams in on the SP HWDGE as five 2-tile (4 MiB-of-bf16) groups,
    then singles/halves/quarters tapering down, every X buffer written
    exactly once (no recycle waits; the Tile framework only has 8 HWDGE
    completion-semaphore lanes, so small late dma_starts would otherwise
    stall behind 2-tile predecessors).
  - per tile: predicated copy overwrites the f0 column with f1 IN PLACE
    (no separate p_win copy); (f0-P1)^2 is split — half columns as two
    DVE bf16 ops, half on the scalar engine — so no engine exceeds the
    DMA rate; Ln on the scalar engine reads the predicated column.
  - matmuls with per-tile stationary columns (1, lo, hi) and their
    NEGATION fold +log(p) and -(f0-P1)^2 into one fp32 PSUM accumulator;
    pos = 128*t + p, and the lo/hi byte split keeps weights exact in
    bf16.
  - the last tile is 4 column-chunks so only a ~256-column chain is
    exposed after the final DMA; its PSUM columns copy out per chunk and
    the output DMA goes in two halves.
  - host folds the per-core [3, F] partials into the closed-form
    weighted sum (weights are affine in (pos, f)) in float64.
"""

import numpy as np
import ml_dtypes
from contextlib import ExitStack

import concourse.bass as bass
import concourse.mybir as mybir
import concourse.tile as tile
from concourse.bass_utils import run_bass_kernel_spmd


MAX_SYNC_WAITS = 1


def _spill_excess_waits(nc, max_waits=MAX_SYNC_WAITS):
    """The walrus in this toolchain rejects instructions carrying more than
    a couple of sync waits ("Too many sync wait commands"). Spill the excess
    onto same-engine NOPs inserted immediately before — semantically
    identical (consecutive sem-ge waits on one engine)."""
    import bass_rust

    k = 0
    for f in nc.m.functions:
        for b in f.blocks:
            out = []
            changed = False
            for inst in b.instructions:
                si = inst.sync_info
                waits = list(si.on_wait or []) if si is not None else []
                if len(waits) > max_waits:
                    chunks = [
                        waits[i : i + max_waits]
                        for i in range(0, len(waits), max_waits)
                    ]
                    for chunk in chunks[:-1]:
                        nop = mybir.InstNoOp(name=f"antspill-{k}", ins=[], outs=[])
                        k += 1
                        nop.engine = inst.engine
                        nop.sync_info = bass_rust.SyncInfo(
                            on_wait=chunk, on_update=[]
                        )
                        out.append(nop)
                    inst.sync_info = bass_rust.SyncInfo(
                        on_wait=chunks[-1], on_update=list(si.on_update or [])
                    )
                    changed = True
                out.append(inst)
            if changed:
                b.instructions = out

N_TOTAL = 16777216
N_CORES = 8
P = 128          # SBUF partitions
F = 1024         # rows per partition per tile
T = 16           # tiles per core; P*F*T = 2097152 = N_TOTAL/N_CORES
CH = 4           # column chunks for the tapered last tile
CF = F // CH
R = 1.0
P1 = 0.5


def build_nc(F=F, T=T):
    M = P * F * T
    nc = bass.Bass(
        "TRN2", target_bir_lowering=False, debug=False,
        enable_asserts=False, num_devices=1,
    )
    fo = nc.dram_tensor("fo", [M, 2], mybir.dt.bfloat16, kind="ExternalInput")
    pv = nc.dram_tensor("pv", [P, T * F], mybir.dt.int8, kind="ExternalInput")
    wt = nc.dram_tensor("wt", [P, 6 * T], mybir.dt.bfloat16, kind="ExternalInput")
    out = nc.dram_tensor("out", [3, F], mybir.dt.float32, kind="ExternalOutput")

    fo_r = fo.ap().rearrange("(t p f) c -> t p f c", t=T, p=P, f=F)
    # 2-tile DMA groups: per partition two contiguous 4 KiB chunks. Fewer,
    # larger dma_starts keep more bytes in flight per HWDGE semaphore lane
    # (the Tile framework rotates over only 8 of them).
    fo_g = fo.ap().rearrange("(u v p f) c -> u p v f c", u=T // 2, v=2, p=P, f=F)

    with tile.TileContext(nc) as tc, ExitStack() as ctx:
        xp = ctx.enter_context(tc.tile_pool(name="xp", bufs=4))
        mp = ctx.enter_context(tc.tile_pool(name="mp", bufs=4))
        cp = ctx.enter_context(tc.tile_pool(name="cp", bufs=1))
        ps = ctx.enter_context(tc.tile_pool(name="ps", bufs=1, space="PSUM"))

        nbias = cp.tile([P, 1], mybir.dt.float32)
        nc.vector.memset(nbias[:], -P1)
        W = cp.tile([P, 6 * T], mybir.dt.bfloat16)
        nc.scalar.dma_start(W[:], wt[:])
        V_all = cp.tile([P, T * F], mybir.dt.int8)
        # four quarter-loads on the otherwise-idle Activation HWDGE: the
        # first quarter lands fast (doesn't sit behind 16 KiB descriptor
        # trains), unblocking the first tiles' predicated copies early
        pv_q = pv.ap().rearrange("p (q c) -> q p c", q=4)
        for q in range(4):
            nc.scalar.dma_start(V_all[:, q * (T * F // 4) : (q + 1) * (T * F // 4)], pv_q[q])
        acc = ps.tile([3, F], mybir.dt.float32)
        ob = cp.tile([3, F], mybir.dt.float32)

        def chunk_load(t, h0, w, suf, bufs):
            """DMA + elementwise for a column chunk of tile t; mms emitted
            separately so PE accumulation order is preserved."""
            sl = slice(h0, h0 + w)
            Xc = xp.tile([P, w, 2], mybir.dt.bfloat16, tag=f"Xc{suf}", bufs=bufs,
                         name=f"Xc_{t}_{h0}")
            nc.sync.dma_start(Xc[:], fo_r[t, :, sl])
            sqc = mp.tile([P, w], mybir.dt.bfloat16, tag=f"sqc{suf}", bufs=bufs,
                          name=f"sqc_{t}_{h0}")
            nc.scalar.activation(
                sqc[:], Xc[:, :, 0], mybir.ActivationFunctionType.Square,
                bias=nbias[:],
            )
            nc.vector.copy_predicated(
                Xc[:, :, 0], V_all[:, t * F + h0 : t * F + h0 + w], Xc[:, :, 1]
            )
            lpc = mp.tile([P, w], mybir.dt.bfloat16, tag=f"lpc{suf}", bufs=bufs,
                          name=f"lpc_{t}_{h0}")
            nc.scalar.activation(lpc[:], Xc[:, :, 0], mybir.ActivationFunctionType.Ln)
            return lpc, sqc

        # full tiles 0..T-7 in groups of 2 per dma_start
        def full_tile(t, Xv, Xsq):
            """p_win in place: overwrite the f0 column with f1 where the
            winner index is 1. The squares read f0 first (WAR dep inserted
            by the tile framework); Ln then reads the predicated column."""
            nc.vector.copy_predicated(
                Xv, V_all[:, t * F : (t + 1) * F], Xsq
            )
            return Xv

        def square_split(t, Xf0):
            """(f0-P1)^2 -> bf16; cols [0:HS) via two DVE bf16 ops, the rest
            on the scalar engine, so neither engine exceeds the DMA rate."""
            HS = 512
            sq = mp.tile([P, F], mybir.dt.bfloat16, tag="sq", bufs=4,
                         name=f"sq_{t}")
            tmp = mp.tile([P, HS], mybir.dt.bfloat16, tag="sqtmp", bufs=2,
                          name=f"sqtmp_{t}")
            nc.vector.tensor_scalar_add(tmp[:], Xf0[:, 0:HS], -P1)
            nc.vector.tensor_tensor(
                sq[:, 0:HS], tmp[:], tmp[:], op=mybir.AluOpType.mult
            )
            nc.scalar.activation(
                sq[:, HS:F], Xf0[:, HS:F],
                mybir.ActivationFunctionType.Square, bias=nbias[:],
            )
            return sq

        def tile_tail(t, pw, sq):
            lp = mp.tile([P, F], mybir.dt.bfloat16, tag="lp", bufs=4, name=f"lp_{t}")
            nc.scalar.activation(lp[:], pw, mybir.ActivationFunctionType.Ln)
            for h0 in range(0, F, 512):
                sl = slice(h0, h0 + 512)
                nc.tensor.matmul(
                    acc[:, sl], W[:, 6 * t : 6 * t + 3], lp[:, sl],
                    start=(t == 0), stop=False,
                )
                nc.tensor.matmul(
                    acc[:, sl], W[:, 6 * t + 3 : 6 * t + 6], sq[:, sl],
                    start=False, stop=False,
                )

        NG = 5   # groups of 2 full tiles: t0..t9 (t10..15 taper down)
        for g in range(NG):
            X2 = xp.tile([P, 2, F, 2], mybir.dt.bfloat16, tag="X2", bufs=5)
            nc.sync.dma_start(X2[:], fo_g[g])
            sqs = [
                square_split(2 * g + v, X2[:, v, :, 0]) for v in range(2)
            ]
            pws = [
                full_tile(2 * g + v, X2[:, v, :, 0], X2[:, v, :, 1])
                for v in range(2)
            ]
            for v in range(2):
                tile_tail(2 * g + v, pws[v], sqs[v])

        # taper: singles t10..13 (own buffers, no recycling)
        for t in range(2 * NG, T - 2):
            X = xp.tile([P, F, 2], mybir.dt.bfloat16, tag="X1", bufs=4,
                        name=f"X_{t}")
            nc.sync.dma_start(X[:], fo_r[t])
            sq = square_split(t, X[:, :, 0])
            pw = full_tile(t, X[:, :, 0], X[:, :, 1])
            tile_tail(t, pw, sq)

        # halves of t14
        t = T - 2
        for h0 in range(0, F, 512):
            sl = slice(h0, h0 + 512)
            lpc, sqc = chunk_load(t, h0, 512, "h", 2)
            nc.tensor.matmul(
                acc[:, sl], W[:, 6 * t : 6 * t + 3], lpc[:],
                start=False, stop=False,
            )
            nc.tensor.matmul(
                acc[:, sl], W[:, 6 * t + 3 : 6 * t + 6], sqc[:],
                start=False, stop=False,
            )

        # quarter chunks of the last tile: they land last, with only a
        # ~256-column chain exposed behind them
        t = T - 1
        for c in range(CH):
            h0 = c * CF
            sl = slice(h0, h0 + CF)
            lpc, sqc = chunk_load(t, h0, CF, "q", CH)
            nc.tensor.matmul(
                acc[:, sl], W[:, 6 * t : 6 * t + 3], lpc[:],
                start=False, stop=False,
            )
            nc.tensor.matmul(
                acc[:, sl], W[:, 6 * t + 3 : 6 * t + 6], sqc[:],
                start=False, stop=True,
            )
            nc.vector.tensor_copy(ob[:, sl], acc[:, sl])
            if c == 1:
                nc.sync.dma_start(out[:, 0:512], ob[:, 0:512])
        nc.sync.dma_start(out[:, 512:F], ob[:, 512:F])
    _spill_excess_waits(nc)
    return nc


def build_wt(T=T):
    """Per-tile stationary matrix: columns (1, pos_lo, pos_hi, -1, -pos_lo,
    -pos_hi), where pos = 128*t + p is the row-chunk index. The negated
    triple multiplies the Square term so PSUM performs log(p) - (f0-P1)^2.
    lo/hi split keeps values exact in bf16 (lo < 256; hi a multiple of 256)."""
    cols = np.zeros((P, 6 * T), np.float32)
    p_idx = np.arange(P, dtype=np.int64)
    for t in range(T):
        pos = t * P + p_idx
        lo = pos & 255
        hi = pos - lo
        cols[:, 6 * t] = 1.0
        cols[:, 6 * t + 1] = lo
        cols[:, 6 * t + 2] = hi
        cols[:, 6 * t + 3] = -1.0
        cols[:, 6 * t + 4] = -lo
        cols[:, 6 * t + 5] = -hi
    return cols.astype(ml_dtypes.bfloat16)


def combine(outs, F=F, T=T):
    """Fold per-core [3, F] partials into the loss.

    Row i = k*M + pos*F + f. Per core:
      sum_i per_i * i = k*M*S + F*(sum pos*per) + (sum f*per)
    with S = sum(c0), sum pos*per = sum(c_lo + c_hi), sum f*per = sum(f*c0).
    """
    M = P * F * T
    n = M * len(outs)
    # mirror the reference's fp32 denom computation
    denom = float(np.float32(n) * np.float32(n + 1) * np.float32(0.5))
    j = np.arange(F, dtype=np.float64)
    total = 0.0
    for k, o in enumerate(outs):
        c0 = o[0].astype(np.float64)
        cw = o[1].astype(np.float64) + o[2].astype(np.float64)
        total += (k * M) * c0.sum() + F * cw.sum() + (j * c0).sum()
    return -total / denom


_NC_CACHE = {}


def _run(final_out, point_victor, **spmd_kwargs):
    fo = np.ascontiguousarray(np.asarray(final_out, dtype=np.float32).astype(ml_dtypes.bfloat16))
    pv = np.asarray(point_victor)
    assert fo.shape == (N_TOTAL, 2) and pv.shape == (N_TOTAL,)
    M = N_TOTAL // N_CORES
    # pack the 0/1 winner indices to int8 and pre-transpose per core to
    # [P, T*F] (partition-major) so the kernel loads all of pv in one DMA
    # with 16 KiB descriptors — 4x less HBM traffic than int32
    pv8 = pv.astype(np.int8).reshape(N_CORES, T, P, F).transpose(0, 2, 1, 3)
    pv8 = np.ascontiguousarray(pv8).reshape(N_CORES, P, T * F)

    if "nc" not in _NC_CACHE:
        _NC_CACHE["nc"] = build_nc()
    nc = _NC_CACHE["nc"]
    wt = build_wt()

    in_maps = [
        {"fo": fo[k * M : (k + 1) * M], "pv": pv8[k], "wt": wt}
        for k in range(N_CORES)
    ]
    res = run_bass_kernel_spmd(nc, in_maps, core_ids=list(range(N_CORES)), **spmd_kwargs)
    outs = [r["out"] for r in res.results]
    return np.float32(combine(outs)), res


def kernel(final_out, point_victor):
    return _run(final_out, point_victor)[0]

